# revision 15
# baseline (speedup 1.0000x reference)
"""DRQConv2d (dual-region quantized conv) Trainium2 kernel.

Reference semantics (see problem statement):
  mask  = upsample8(avgpool8(x) >= 0.05)             per (b, c)
  xh    = where(mask, x, 1e-5);  xl = where(mask, 1e-5, x)
  qh    = clip(round(xh/sh), 0, 255) * sh            (uint8 fake-quant)
  ql    = clip(round(xl/sl), 0, 15) * sl             (uint4 fake-quant)
  qwh   = per-oc quant of w_high to +-127,  qwl = per-oc quant of w_low to +-7
  y     = conv3x3(qh, qwh) + conv3x3(ql, qwl)        (pad 1)

Key facts exploited here:
  * 1e-5 quantizes to exactly 0 on both paths, so the masked fill is just a
    multiply by the {0,1} mask after rounding.
  * The quantized activations and weights are exact small integers
    (<=255 / <=127), which bf16 represents exactly; PSUM accumulates fp32.
    So bf16 matmuls reproduce the fp32 reference up to accumulation order.
  * conv3x3 = 9 shift-offset matmuls (K=C_in=128) accumulated in PSUM over a
    zero-padded 58x58 activation layout.

Sharding: data-parallel over batch. 32 images -> 4 per core on 8 cores,
weights replicated; outputs concatenated on host. No collectives.
"""

import numpy as np

P = 128            # channels (both in and out) == partitions
B_TOTAL = 32
N_CORES = 8
BPC = B_TOTAL // N_CORES   # images per core
H = W = 56
HP = WP = H + 2    # zero-padded layout
NPIX = H * W       # 3136
NPAD = HP * WP     # 3364
NTAPS = 9
ROWS_PER_CHUNK = 8
NCHUNK = H // ROWS_PER_CHUNK          # 7
NFREE = ROWS_PER_CHUNK * W            # 448 columns per matmul
MAGIC = float(np.float32(1.5 * 2 ** 23))   # fp32 round-to-nearest magic
POOL_K = 8
THRESH = 0.05


def build_program(nc, tc, aps, inv_sh, inv_sl, c_svh, c_svl, bpc=BPC):
    """Emit the whole per-core program inside an open TileContext.

    aps: dict with DRAM APs: x [bpc,P,NPIX], w_high [P,1152], w_low [P,1152],
         y [bpc,P,NPIX].
    inv_sh/inv_sl: 1/act_scale (host floats, baked as immediates).
    c_svh/c_svl: act_scale / (2^(b-1)-1) -- multiplied by per-oc |w|max to give
         the combined output scale.
    """
    import concourse.mybir as mybir
    from concourse.alu_op_type import AluOpType as op
    from concourse.masks import make_identity

    f32 = mybir.dt.float32
    bf16 = mybir.dt.bfloat16
    X = mybir.AxisListType.X

    x_d, wh_d, wl_d, y_d = aps["x"], aps["w_high"], aps["w_low"], aps["y"]

    sum_thresh = float(np.float32(THRESH) * POOL_K * POOL_K)  # exact pow2 scale

    with (
        tc.tile_pool(name="consts", bufs=1) as consts,
        tc.tile_pool(name="wtmp", bufs=2) as wtmp_pool,
        tc.tile_pool(name="tp_psum", bufs=1, space="PSUM") as tp_psum,
        tc.tile_pool(name="acts", bufs=2) as acts,
        tc.tile_pool(name="masks", bufs=2) as maskp,
        tc.tile_pool(name="qtiles", bufs=4) as qtiles,
        tc.tile_pool(name="outs", bufs=2) as outs_pool,
        tc.tile_pool(name="conv_psum", bufs=7, space="PSUM") as conv_psum,
    ):
        identity = consts.tile([P, P], f32)
        make_identity(nc, identity[:])

        qwt = {}   # conv -> bf16 [P(ic), 9*P(oc)] transposed integer weights
        sv = {}    # conv -> f32 [P(oc), 1] combined output scale

        def weight_prep(conv, w_dram, nw, c_sv, scale_by_ratio):
            """Quantize weights per-oc; 'l' weights additionally pre-scaled by
            sv_l/sv_h so the low conv can accumulate into the high conv's PSUM
            (single final scale by sv_h)."""
            wnat = wtmp_pool.tile([P, P * NTAPS], f32, tag="wnat")
            nc.sync.dma_start(out=wnat[:], in_=w_dram)
            absmax = consts.tile([P, 1], f32, tag=f"absmax_{conv}")
            nc.vector.tensor_reduce(
                absmax[:], wnat[:], axis=X, op=op.max, apply_absolute_value=True
            )
            sv_t = consts.tile([P, 1], f32, tag=f"sv_{conv}")
            nc.vector.tensor_scalar_mul(sv_t[:], absmax[:], c_sv)
            sv[conv] = sv_t
            rcp = consts.tile([P, 1], f32, tag=f"rcp_{conv}")
            nc.vector.reciprocal(rcp[:], absmax[:])
            rs = consts.tile([P, 1], f32, tag=f"rs_{conv}")
            nc.vector.tensor_scalar_mul(rs[:], rcp[:], nw)

            # integer-quantize in natural [oc, ic*9] layout (per-partition scalar)
            wq = wtmp_pool.tile([P, P * NTAPS], f32, tag="wq")
            nc.vector.tensor_scalar(
                wq[:], wnat[:], rs[:, 0:1], MAGIC, op0=op.mult, op1=op.add
            )
            nc.vector.tensor_scalar(
                wq[:], wq[:], MAGIC, nw, op0=op.subtract, op1=op.min
            )
            nc.vector.tensor_scalar_max(wq[:], wq[:], -nw)
            if scale_by_ratio:
                rcp_svh = consts.tile([P, 1], f32)
                nc.vector.reciprocal(rcp_svh[:], sv["h"][:, 0:1])
                ratio = consts.tile([P, 1], f32)
                nc.vector.tensor_tensor(
                    ratio[:], sv_t[:], rcp_svh[:], op=op.mult
                )
                nc.vector.tensor_scalar_mul(wq[:], wq[:], ratio[:, 0:1])

            # transpose each 3x3 tap: [oc, ic] -> [ic, oc], cast to bf16
            qwt_t = consts.tile([P, NTAPS * P], bf16, tag=f"qwt_{conv}")
            wq_v = wq[:].rearrange("p (i t) -> p t i", t=NTAPS)
            for tap in range(NTAPS):
                tp = tp_psum.tile([P, P], f32, tag="tp")
                nc.tensor.transpose(tp[:], wq_v[:, tap, :], identity[:])
                nc.vector.tensor_copy(
                    out=qwt_t[:, tap * P:(tap + 1) * P], in_=tp[:]
                )
            qwt[conv] = qwt_t

        def mask_prep(b, xt):
            """Block sums -> threshold -> full-res {0,1} masks [P, NPIX]."""
            r1 = acts.tile([P, H * NCHUNK], f32, tag="r1")   # [P, 392]
            nc.vector.reduce_sum(
                r1[:], xt[:].rearrange("p (r c) -> p r c", c=POOL_K), axis=X
            )
            # r1 is (h, wb)-ordered; h-blocks are stride-7 so walrus's
            # 3-AP-dim limit forces one small reduce per row-block.
            r2 = acts.tile([P, NCHUNK * NCHUNK], f32, tag="r2")  # [P, 49]
            r1v = r1[:].rearrange(
                "p (a b c) -> p a b c", a=NCHUNK, b=POOL_K, c=NCHUNK
            )
            for hb in range(NCHUNK):
                nc.vector.reduce_sum(
                    r2[:, hb * NCHUNK:(hb + 1) * NCHUNK],
                    r1v[:, hb].rearrange("p b c -> p c b"),
                    axis=X,
                )
            m = acts.tile([P, NCHUNK * NCHUNK], f32, tag="m")
            nc.vector.tensor_scalar(
                m[:], r2[:], sum_thresh, None, op0=op.is_ge
            )
            # expand to full res: [P,49] -> [P,392] (DVE) -> [P,3136] (ACT)
            mr = acts.tile([P, NCHUNK * W], f32, tag="mr")
            nc.vector.tensor_copy(
                out=mr[:].rearrange("p (r c) -> p r c", c=POOL_K),
                in_=m[:].unsqueeze(2).broadcast_to((P, NCHUNK * NCHUNK, POOL_K)),
            )
            mexp_h = maskp.tile([P, NPIX], f32, tag="mexp_h")
            mh3 = mexp_h[:].rearrange("p (r c) -> p r c", r=H)
            for hb in range(NCHUNK):
                nc.scalar.copy(
                    out=mh3[:, hb * POOL_K:(hb + 1) * POOL_K, :],
                    in_=mr[:, hb * W:(hb + 1) * W]
                    .unsqueeze(1).broadcast_to((P, POOL_K, W)),
                )
            mexp_l = maskp.tile([P, NPIX], f32, tag="mexp_l")
            nc.scalar.activation(
                mexp_l[:], mexp_h[:], mybir.ActivationFunctionType.Copy,
                bias=1.0, scale=-1.0,
            )
            return {"h": mexp_h, "l": mexp_l}

        def quant_act(b, xt, mexp, conv, inv_s, qmax):
            """relu/scale (ACT) -> min+round (DVE) -> mask-mult into padded
            bf16 tile."""
            r = acts.tile([P, NPIX], f32, tag="r")
            nc.scalar.activation(
                r[:], xt[:], mybir.ActivationFunctionType.Relu, scale=inv_s
            )
            t = acts.tile([P, NPIX], f32, tag="t")
            nc.vector.tensor_scalar(
                t[:], r[:], qmax, MAGIC, op0=op.min, op1=op.add
            )
            q = qtiles.tile([P, NPAD], bf16, tag="q")
            q2 = q[:].rearrange("p (r c) -> p r c", r=HP)
            # zero borders: rows 0,57 and cols 0,57 (gpsimd; keeps DVE free)
            nc.gpsimd.memset(q2[:, 0:HP:HP - 1, :], 0.0)
            nc.gpsimd.memset(q2[:, 1:HP - 1, 0:WP:WP - 1], 0.0)
            nc.vector.scalar_tensor_tensor(
                out=q2[:, 1:H + 1, 1:W + 1],
                in0=t[:].rearrange("p (r c) -> p r c", r=H),
                scalar=MAGIC,
                in1=mexp[conv][:].rearrange("p (r c) -> p r c", r=H),
                op0=op.subtract, op1=op.mult,
            )
            return q

        def conv_pass(qa_tile, conv, pss, first):
            """One full conv pass (9 taps x 7 chunks) accumulating into the
            7 live PSUM banks. Low weights are pre-scaled by sv_l/sv_h so both
            passes share banks and a single evacuation."""
            q2 = qa_tile[:].rearrange("p (r c) -> p r c", r=HP)
            for c in range(NCHUNK):
                r0 = c * ROWS_PER_CHUNK
                for tap in range(NTAPS):
                    kh, kw = divmod(tap, 3)
                    rhs = q2[:, r0 + kh:r0 + kh + ROWS_PER_CHUNK, kw:kw + W]
                    nc.tensor.matmul(
                        pss[c][:], qwt[conv][:, tap * P:(tap + 1) * P], rhs,
                        start=(first and tap == 0),
                        stop=(not first and tap == NTAPS - 1),
                    )

        def conv_image(b, qa):
            acc = outs_pool.tile([P, NPIX], f32, tag="acc")
            pss = [conv_psum.tile([P, NFREE], f32, tag="ps", name=f"ps{b}_{c}")
                   for c in range(NCHUNK)]
            conv_pass(qa["h"], "h", pss, True)
            conv_pass(qa["l"], "l", pss, False)
            for c in range(NCHUNK):
                r0 = c * ROWS_PER_CHUNK
                seg = acc[:, r0 * W:(r0 + ROWS_PER_CHUNK) * W]
                nc.vector.tensor_scalar_mul(seg, pss[c][:], sv["h"][:, 0:1])
            nc.sync.dma_start(out=y_d[b], in_=acc[:])

        # ---------------- schedule ----------------
        # weights-h first, then image 0's high-path quant, then weights-l,
        # so image 0's convs can start while the tail of setup still runs.
        weight_prep("h", wh_d, 127.0, c_svh, False)

        xts = {}
        xts[0] = acts.tile([P, NPIX], f32, tag="xt", name="xt0")
        nc.sync.dma_start(out=xts[0][:], in_=x_d[0])
        mexp0 = mask_prep(0, xts[0])
        qa0 = {"h": quant_act(0, xts[0], mexp0, "h", inv_sh, 255.0)}

        weight_prep("l", wl_d, 7.0, c_svl, True)
        qa0["l"] = quant_act(0, xts[0], mexp0, "l", inv_sl, 15.0)
        conv_image(0, qa0)

        for b in range(1, bpc):
            xt = acts.tile([P, NPIX], f32, tag="xt", name=f"xt{b}")
            nc.sync.dma_start(out=xt[:], in_=x_d[b])
            mexp = mask_prep(b, xt)
            qa = {
                "h": quant_act(b, xt, mexp, "h", inv_sh, 255.0),
                "l": quant_act(b, xt, mexp, "l", inv_sl, 15.0),
            }
            conv_image(b, qa)


def make_bass(inv_sh, inv_sl, c_svh, c_svl, bpc=BPC):
    import concourse.bacc as bacc
    import concourse.mybir as mybir
    from concourse.tile import TileContext

    f32 = mybir.dt.float32
    nc = bacc.Bacc("TRN2", debug=False)
    x = nc.dram_tensor("x", [bpc, P, NPIX], f32, kind="ExternalInput")
    wh = nc.dram_tensor("w_high", [P, P * NTAPS], f32, kind="ExternalInput")
    wl = nc.dram_tensor("w_low", [P, P * NTAPS], f32, kind="ExternalInput")
    y = nc.dram_tensor("y", [bpc, P, NPIX], f32, kind="ExternalOutput")
    aps = {"x": x.ap(), "w_high": wh.ap(), "w_low": wl.ap(), "y": y.ap()}
    with TileContext(nc) as tc:
        build_program(nc, tc, aps, inv_sh, inv_sl, c_svh, c_svl, bpc=bpc)
    nc.compile()
    return nc


def _scale_consts(act_scale_high, act_scale_low):
    sh = float(np.float32(act_scale_high))
    sl = float(np.float32(act_scale_low))
    inv_sh = float(np.float32(1.0 / np.float64(sh)))
    inv_sl = float(np.float32(1.0 / np.float64(sl)))
    c_svh = float(np.float32(np.float64(sh) / 127.0))
    c_svl = float(np.float32(np.float64(sl) / 7.0))
    return inv_sh, inv_sl, c_svh, c_svl


def _run(x, w_high, w_low, act_scale_high, act_scale_low, trace=False, **kw):
    from concourse import bass_utils

    x = np.ascontiguousarray(np.asarray(x, dtype=np.float32))
    w_high = np.ascontiguousarray(np.asarray(w_high, dtype=np.float32))
    w_low = np.ascontiguousarray(np.asarray(w_low, dtype=np.float32))

    inv_sh, inv_sl, c_svh, c_svl = _scale_consts(act_scale_high, act_scale_low)
    nc = make_bass(inv_sh, inv_sl, c_svh, c_svl)

    wh_flat = w_high.reshape(P, P * NTAPS)
    wl_flat = w_low.reshape(P, P * NTAPS)
    in_maps = []
    for core in range(N_CORES):
        xs = x[core * BPC:(core + 1) * BPC].reshape(BPC, P, NPIX)
        in_maps.append(
            {
                "x": np.ascontiguousarray(xs),
                "w_high": wh_flat,
                "w_low": wl_flat,
            }
        )
    res = bass_utils.run_bass_kernel_spmd(
        nc, in_maps, core_ids=list(range(N_CORES)), trace=trace, **kw
    )
    y = np.concatenate([r["y"].reshape(BPC, P, H, W) for r in res.results], axis=0)
    return y, res


def kernel(x, w_high, w_low, act_scale_high, act_scale_low):
    y, _ = _run(x, w_high, w_low, act_scale_high, act_scale_low)
    return y


# revision 19
# speedup vs baseline: 1.0018x; 1.0018x over previous
"""DRQConv2d (dual-region quantized conv) Trainium2 kernel.

Reference semantics (see problem statement):
  mask  = upsample8(avgpool8(x) >= 0.05)             per (b, c)
  xh    = where(mask, x, 1e-5);  xl = where(mask, 1e-5, x)
  qh    = clip(round(xh/sh), 0, 255) * sh            (uint8 fake-quant)
  ql    = clip(round(xl/sl), 0, 15) * sl             (uint4 fake-quant)
  qwh   = per-oc quant of w_high to +-127,  qwl = per-oc quant of w_low to +-7
  y     = conv3x3(qh, qwh) + conv3x3(ql, qwl)        (pad 1)

Key facts exploited here:
  * 1e-5 quantizes to exactly 0 on both paths, so the masked fill is just a
    multiply by the {0,1} mask after rounding.
  * The quantized activations and weights are exact small integers
    (<=255 / <=127), which bf16 represents exactly; PSUM accumulates fp32.
    So bf16 matmuls reproduce the fp32 reference up to accumulation order.
  * conv3x3 = 9 shift-offset matmuls (K=C_in=128) accumulated in PSUM over a
    zero-padded 58x58 activation layout.

Sharding: data-parallel over batch. 32 images -> 4 per core on 8 cores,
weights replicated; outputs concatenated on host. No collectives.
"""

import numpy as np

P = 128            # channels (both in and out) == partitions
B_TOTAL = 32
N_CORES = 8
BPC = B_TOTAL // N_CORES   # images per core
H = W = 56
HP = WP = H + 2    # zero-padded layout
NPIX = H * W       # 3136
NPAD = HP * WP     # 3364
NTAPS = 9
ROWS_PER_CHUNK = 8
NCHUNK = H // ROWS_PER_CHUNK          # 7
NFREE = ROWS_PER_CHUNK * W            # 448 columns per matmul
MAGIC = float(np.float32(1.5 * 2 ** 23))   # fp32 round-to-nearest magic
POOL_K = 8
THRESH = 0.05


def build_program(nc, tc, aps, inv_sh, inv_sl, c_svh, c_svl, bpc=BPC):
    """Emit the whole per-core program inside an open TileContext.

    aps: dict with DRAM APs: x [bpc,P,NPIX], w_high [P,1152], w_low [P,1152],
         y [bpc,P,NPIX].
    inv_sh/inv_sl: 1/act_scale (host floats, baked as immediates).
    c_svh/c_svl: act_scale / (2^(b-1)-1) -- multiplied by per-oc |w|max to give
         the combined output scale.
    """
    import concourse.mybir as mybir
    from concourse.alu_op_type import AluOpType as op
    from concourse.masks import make_identity

    f32 = mybir.dt.float32
    bf16 = mybir.dt.bfloat16
    X = mybir.AxisListType.X

    x_d, wh_d, wl_d, y_d = aps["x"], aps["w_high"], aps["w_low"], aps["y"]

    sum_thresh = float(np.float32(THRESH) * POOL_K * POOL_K)  # exact pow2 scale

    with (
        tc.tile_pool(name="consts", bufs=1) as consts,
        tc.tile_pool(name="wtmp", bufs=2) as wtmp_pool,
        tc.tile_pool(name="tp_psum", bufs=1, space="PSUM") as tp_psum,
        tc.tile_pool(name="acts", bufs=2) as acts,
        tc.tile_pool(name="masks", bufs=2) as maskp,
        tc.tile_pool(name="qtiles", bufs=4) as qtiles,
        tc.tile_pool(name="outs", bufs=2) as outs_pool,
        tc.tile_pool(name="conv_psum", bufs=7, space="PSUM") as conv_psum,
    ):
        identity = consts.tile([P, P], f32)
        make_identity(nc, identity[:])

        qwt = {}   # conv -> bf16 [P(ic), 9*P(oc)] transposed integer weights
        sv = {}    # conv -> f32 [P(oc), 1] combined output scale

        def weight_prep(conv, w_dram, nw, c_sv, scale_by_ratio):
            """Quantize weights per-oc; 'l' weights additionally pre-scaled by
            sv_l/sv_h so the low conv can accumulate into the high conv's PSUM
            (single final scale by sv_h)."""
            wnat = wtmp_pool.tile([P, P * NTAPS], f32, tag="wnat")
            nc.sync.dma_start(out=wnat[:], in_=w_dram)
            absmax = consts.tile([P, 1], f32, tag=f"absmax_{conv}")
            nc.vector.tensor_reduce(
                absmax[:], wnat[:], axis=X, op=op.max, apply_absolute_value=True
            )
            sv_t = consts.tile([P, 1], f32, tag=f"sv_{conv}")
            nc.vector.tensor_scalar_mul(sv_t[:], absmax[:], c_sv)
            sv[conv] = sv_t
            rcp = consts.tile([P, 1], f32, tag=f"rcp_{conv}")
            nc.vector.reciprocal(rcp[:], absmax[:])
            rs = consts.tile([P, 1], f32, tag=f"rs_{conv}")
            nc.vector.tensor_scalar_mul(rs[:], rcp[:], nw)

            # integer-quantize in natural [oc, ic*9] layout (per-partition scalar)
            wq = wtmp_pool.tile([P, P * NTAPS], f32, tag="wq")
            nc.vector.tensor_scalar(
                wq[:], wnat[:], rs[:, 0:1], MAGIC, op0=op.mult, op1=op.add
            )
            nc.vector.tensor_scalar(
                wq[:], wq[:], MAGIC, nw, op0=op.subtract, op1=op.min
            )
            nc.vector.tensor_scalar_max(wq[:], wq[:], -nw)
            if scale_by_ratio:
                rcp_svh = consts.tile([P, 1], f32)
                nc.vector.reciprocal(rcp_svh[:], sv["h"][:, 0:1])
                ratio = consts.tile([P, 1], f32)
                nc.vector.tensor_tensor(
                    ratio[:], sv_t[:], rcp_svh[:], op=op.mult
                )
                nc.vector.tensor_scalar_mul(wq[:], wq[:], ratio[:, 0:1])

            # transpose each 3x3 tap: [oc, ic] -> [ic, oc], cast to bf16
            qwt_t = consts.tile([P, NTAPS * P], bf16, tag=f"qwt_{conv}")
            wq_v = wq[:].rearrange("p (i t) -> p t i", t=NTAPS)
            for tap in range(NTAPS):
                tp = tp_psum.tile([P, P], f32, tag="tp")
                nc.tensor.transpose(tp[:], wq_v[:, tap, :], identity[:])
                nc.vector.tensor_copy(
                    out=qwt_t[:, tap * P:(tap + 1) * P], in_=tp[:]
                )
            qwt[conv] = qwt_t

        def mask_prep(b, xt):
            """Block sums -> threshold -> full-res {0,1} masks [P, NPIX].

            w-blocksums via a pairwise add tree on gpsimd (frees DVE); the
            h-blocksum is one DVE reduce producing a (wb, hb)-ordered layout
            (so the reduce axis is contiguous), fixed up by a tiny transposing
            copy after the threshold."""
            s1 = acts.tile([P, NPIX // 2], f32, tag="s1", bufs=1)
            x2 = xt[:].rearrange("p (r c) -> p r c", c=2)
            nc.gpsimd.tensor_tensor(s1[:], x2[:, :, 0], x2[:, :, 1], op=op.add)
            s2 = acts.tile([P, NPIX // 4], f32, tag="s2", bufs=1)
            s1v = s1[:].rearrange("p (r c) -> p r c", c=2)
            nc.gpsimd.tensor_tensor(s2[:], s1v[:, :, 0], s1v[:, :, 1], op=op.add)
            # last tree level writes r1 TRANSPOSED to (wb, h) order so the
            # h-block reduce is contiguous: index = wb*56 + hb*8 + hi.
            r1 = acts.tile([P, H * NCHUNK], f32, tag="r1")   # [P, 392] (wb, h)
            s2v = s2[:].rearrange("p (r c) -> p r c", c=2)
            r1t = r1[:].rearrange("p (w h) -> p h w", w=NCHUNK)
            nc.gpsimd.tensor_tensor(r1t, s2v[:, :, 0], s2v[:, :, 1], op=op.add)
            r2 = acts.tile([P, NCHUNK * NCHUNK], f32, tag="r2")  # [P,49] (wb,hb)
            nc.vector.reduce_sum(
                r2[:], r1[:].rearrange("p (g c) -> p g c", c=POOL_K), axis=X
            )
            mt = acts.tile([P, NCHUNK * NCHUNK], f32, tag="mt")
            nc.vector.tensor_scalar(
                mt[:], r2[:], sum_thresh, None, op0=op.is_ge
            )
            # fix-up to (hb, wb) order with a tiny transposing copy
            m = acts.tile([P, NCHUNK * NCHUNK], f32, tag="m")
            nc.vector.tensor_copy(
                out=m[:], in_=mt[:].rearrange("p (w h) -> p h w", w=NCHUNK)
            )
            # expand to full res: [P,49] -> [P,392] (DVE) -> [P,3136] (ACT)
            mr = acts.tile([P, NCHUNK * W], f32, tag="mr")
            nc.vector.tensor_copy(
                out=mr[:].rearrange("p (r c) -> p r c", c=POOL_K),
                in_=m[:].unsqueeze(2).broadcast_to((P, NCHUNK * NCHUNK, POOL_K)),
            )
            mexp_h = maskp.tile([P, NPIX], f32, tag="mexp_h")
            mh3 = mexp_h[:].rearrange("p (r c) -> p r c", r=H)
            for hb in range(NCHUNK):
                nc.scalar.copy(
                    out=mh3[:, hb * POOL_K:(hb + 1) * POOL_K, :],
                    in_=mr[:, hb * W:(hb + 1) * W]
                    .unsqueeze(1).broadcast_to((P, POOL_K, W)),
                )
            mexp_l = maskp.tile([P, NPIX], f32, tag="mexp_l")
            nc.vector.tensor_scalar(
                mexp_l[:], mexp_h[:], -1.0, 1.0, op0=op.mult, op1=op.add
            )
            return {"h": mexp_h, "l": mexp_l}

        def quant_act(b, xt, mexp, conv, inv_s, qmax):
            """relu/scale (ACT) -> min+round (DVE) -> mask-mult into padded
            bf16 tile."""
            r = acts.tile([P, NPIX], f32, tag="r")
            nc.scalar.activation(
                r[:], xt[:], mybir.ActivationFunctionType.Relu, scale=inv_s
            )
            t = r
            nc.vector.tensor_scalar(
                t[:], r[:], qmax, MAGIC, op0=op.min, op1=op.add
            )
            q = qtiles.tile([P, NPAD], bf16, tag="q")
            q2 = q[:].rearrange("p (r c) -> p r c", r=HP)
            # zero borders: rows 0,57 and cols 0,57 (gpsimd; keeps DVE free)
            nc.gpsimd.memset(q2[:, 0:HP:HP - 1, :], 0.0)
            nc.gpsimd.memset(q2[:, 1:HP - 1, 0:WP:WP - 1], 0.0)
            nc.vector.scalar_tensor_tensor(
                out=q2[:, 1:H + 1, 1:W + 1],
                in0=t[:].rearrange("p (r c) -> p r c", r=H),
                scalar=MAGIC,
                in1=mexp[conv][:].rearrange("p (r c) -> p r c", r=H),
                op0=op.subtract, op1=op.mult,
            )
            return q

        def conv_pass(qa_tile, conv, pss, first):
            """One full conv pass (9 taps x 7 chunks) accumulating into the
            7 live PSUM banks. Low weights are pre-scaled by sv_l/sv_h so both
            passes share banks and a single evacuation."""
            q2 = qa_tile[:].rearrange("p (r c) -> p r c", r=HP)
            for c in range(NCHUNK):
                r0 = c * ROWS_PER_CHUNK
                for tap in range(NTAPS):
                    kh, kw = divmod(tap, 3)
                    rhs = q2[:, r0 + kh:r0 + kh + ROWS_PER_CHUNK, kw:kw + W]
                    nc.tensor.matmul(
                        pss[c][:], qwt[conv][:, tap * P:(tap + 1) * P], rhs,
                        start=(first and tap == 0),
                        stop=(not first and tap == NTAPS - 1),
                    )

        def conv_image(b, qa):
            acc = outs_pool.tile([P, NPIX], f32, tag="acc")
            pss = [conv_psum.tile([P, NFREE], f32, tag="ps", name=f"ps{b}_{c}")
                   for c in range(NCHUNK)]
            conv_pass(qa["h"], "h", pss, True)
            conv_pass(qa["l"], "l", pss, False)
            for c in range(NCHUNK):
                r0 = c * ROWS_PER_CHUNK
                seg = acc[:, r0 * W:(r0 + ROWS_PER_CHUNK) * W]
                nc.scalar.mul(seg, pss[c][:], sv["h"][:, 0:1])
                nc.sync.dma_start(
                    out=y_d[b][:, r0 * W:(r0 + ROWS_PER_CHUNK) * W], in_=seg
                )

        # ---------------- schedule ----------------
        # weights-h first, then image 0's high-path quant, then weights-l,
        # so image 0's convs can start while the tail of setup still runs.
        weight_prep("h", wh_d, 127.0, c_svh, False)

        xts = {}
        xts[0] = acts.tile([P, NPIX], f32, tag="xt", name="xt0")
        nc.sync.dma_start(out=xts[0][:], in_=x_d[0])
        mexp0 = mask_prep(0, xts[0])
        qa0 = {"h": quant_act(0, xts[0], mexp0, "h", inv_sh, 255.0)}

        weight_prep("l", wl_d, 7.0, c_svl, True)
        qa0["l"] = quant_act(0, xts[0], mexp0, "l", inv_sl, 15.0)
        conv_image(0, qa0)

        for b in range(1, bpc):
            xt = acts.tile([P, NPIX], f32, tag="xt", name=f"xt{b}")
            nc.sync.dma_start(out=xt[:], in_=x_d[b])
            mexp = mask_prep(b, xt)
            qa = {
                "h": quant_act(b, xt, mexp, "h", inv_sh, 255.0),
                "l": quant_act(b, xt, mexp, "l", inv_sl, 15.0),
            }
            conv_image(b, qa)


def make_bass(inv_sh, inv_sl, c_svh, c_svl, bpc=BPC):
    import concourse.bacc as bacc
    import concourse.mybir as mybir
    from concourse.tile import TileContext

    f32 = mybir.dt.float32
    nc = bacc.Bacc("TRN2", debug=False)
    x = nc.dram_tensor("x", [bpc, P, NPIX], f32, kind="ExternalInput")
    wh = nc.dram_tensor("w_high", [P, P * NTAPS], f32, kind="ExternalInput")
    wl = nc.dram_tensor("w_low", [P, P * NTAPS], f32, kind="ExternalInput")
    y = nc.dram_tensor("y", [bpc, P, NPIX], f32, kind="ExternalOutput")
    aps = {"x": x.ap(), "w_high": wh.ap(), "w_low": wl.ap(), "y": y.ap()}
    with TileContext(nc) as tc:
        build_program(nc, tc, aps, inv_sh, inv_sl, c_svh, c_svl, bpc=bpc)
    nc.compile()
    return nc


def _scale_consts(act_scale_high, act_scale_low):
    sh = float(np.float32(act_scale_high))
    sl = float(np.float32(act_scale_low))
    inv_sh = float(np.float32(1.0 / np.float64(sh)))
    inv_sl = float(np.float32(1.0 / np.float64(sl)))
    c_svh = float(np.float32(np.float64(sh) / 127.0))
    c_svl = float(np.float32(np.float64(sl) / 7.0))
    return inv_sh, inv_sl, c_svh, c_svl


def _run(x, w_high, w_low, act_scale_high, act_scale_low, trace=False, **kw):
    from concourse import bass_utils

    x = np.ascontiguousarray(np.asarray(x, dtype=np.float32))
    w_high = np.ascontiguousarray(np.asarray(w_high, dtype=np.float32))
    w_low = np.ascontiguousarray(np.asarray(w_low, dtype=np.float32))

    inv_sh, inv_sl, c_svh, c_svl = _scale_consts(act_scale_high, act_scale_low)
    nc = make_bass(inv_sh, inv_sl, c_svh, c_svl)

    wh_flat = w_high.reshape(P, P * NTAPS)
    wl_flat = w_low.reshape(P, P * NTAPS)
    in_maps = []
    for core in range(N_CORES):
        xs = x[core * BPC:(core + 1) * BPC].reshape(BPC, P, NPIX)
        in_maps.append(
            {
                "x": np.ascontiguousarray(xs),
                "w_high": wh_flat,
                "w_low": wl_flat,
            }
        )
    res = bass_utils.run_bass_kernel_spmd(
        nc, in_maps, core_ids=list(range(N_CORES)), trace=trace, **kw
    )
    y = np.concatenate([r["y"].reshape(BPC, P, H, W) for r in res.results], axis=0)
    return y, res


def kernel(x, w_high, w_low, act_scale_high, act_scale_low):
    y, _ = _run(x, w_high, w_low, act_scale_high, act_scale_low)
    return y


# revision 21
# speedup vs baseline: 1.0048x; 1.0030x over previous
"""DRQConv2d (dual-region quantized conv) Trainium2 kernel.

Reference semantics (see problem statement):
  mask  = upsample8(avgpool8(x) >= 0.05)             per (b, c)
  xh    = where(mask, x, 1e-5);  xl = where(mask, 1e-5, x)
  qh    = clip(round(xh/sh), 0, 255) * sh            (uint8 fake-quant)
  ql    = clip(round(xl/sl), 0, 15) * sl             (uint4 fake-quant)
  qwh   = per-oc quant of w_high to +-127,  qwl = per-oc quant of w_low to +-7
  y     = conv3x3(qh, qwh) + conv3x3(ql, qwl)        (pad 1)

Key facts exploited here:
  * 1e-5 quantizes to exactly 0 on both paths, so the masked fill is just a
    multiply by the {0,1} mask after rounding.
  * The quantized activations and weights are exact small integers
    (<=255 / <=127), which bf16 represents exactly; PSUM accumulates fp32.
    So bf16 matmuls reproduce the fp32 reference up to accumulation order.
  * conv3x3 = 9 shift-offset matmuls (K=C_in=128) accumulated in PSUM over a
    zero-padded 58x58 activation layout.

Sharding: data-parallel over batch. 32 images -> 4 per core on 8 cores,
weights replicated; outputs concatenated on host. No collectives.
"""

import numpy as np

P = 128            # channels (both in and out) == partitions
B_TOTAL = 32
N_CORES = 8
BPC = B_TOTAL // N_CORES   # images per core
H = W = 56
HP = WP = H + 2    # zero-padded layout
NPIX = H * W       # 3136
NPAD = HP * WP     # 3364
NTAPS = 9
ROWS_PER_CHUNK = 8
NCHUNK = H // ROWS_PER_CHUNK          # 7
NFREE = ROWS_PER_CHUNK * W            # 448 columns per matmul
MAGIC = float(np.float32(1.5 * 2 ** 23))   # fp32 round-to-nearest magic
POOL_K = 8
THRESH = 0.05


def build_program(nc, tc, aps, inv_sh, inv_sl, c_svh, c_svl, bpc=BPC):
    """Emit the whole per-core program inside an open TileContext.

    aps: dict with DRAM APs: x [bpc,P,NPIX], w_high [P,1152], w_low [P,1152],
         y [bpc,P,NPIX].
    inv_sh/inv_sl: 1/act_scale (host floats, baked as immediates).
    c_svh/c_svl: act_scale / (2^(b-1)-1) -- multiplied by per-oc |w|max to give
         the combined output scale.
    """
    import concourse.mybir as mybir
    from concourse.alu_op_type import AluOpType as op
    from concourse.masks import make_identity

    f32 = mybir.dt.float32
    bf16 = mybir.dt.bfloat16
    X = mybir.AxisListType.X

    x_d, wh_d, wl_d, y_d = aps["x"], aps["w_high"], aps["w_low"], aps["y"]

    sum_thresh = float(np.float32(THRESH) * POOL_K * POOL_K)  # exact pow2 scale

    with (
        tc.tile_pool(name="consts", bufs=1) as consts,
        tc.tile_pool(name="wtmp", bufs=2) as wtmp_pool,
        tc.tile_pool(name="tp_psum", bufs=1, space="PSUM") as tp_psum,
        tc.tile_pool(name="acts", bufs=2) as acts,
        tc.tile_pool(name="masks", bufs=2) as maskp,
        tc.tile_pool(name="qtiles", bufs=4) as qtiles,
        tc.tile_pool(name="outs", bufs=2) as outs_pool,
        tc.tile_pool(name="conv_psum", bufs=7, space="PSUM") as conv_psum,
    ):
        identity = consts.tile([P, P], f32)
        make_identity(nc, identity[:])

        qwt = {}   # conv -> bf16 [P(ic), 9*P(oc)] transposed integer weights
        sv = {}    # conv -> f32 [P(oc), 1] combined output scale

        def weight_prep(conv, w_dram, nw, c_sv, scale_by_ratio):
            """Quantize weights per-oc; 'l' weights additionally pre-scaled by
            sv_l/sv_h so the low conv can accumulate into the high conv's PSUM
            (single final scale by sv_h)."""
            wnat = wtmp_pool.tile([P, P * NTAPS], f32, tag="wnat")
            nc.sync.dma_start(out=wnat[:], in_=w_dram)
            absmax = consts.tile([P, 1], f32, tag=f"absmax_{conv}")
            nc.vector.tensor_reduce(
                absmax[:], wnat[:], axis=X, op=op.max, apply_absolute_value=True
            )
            sv_t = consts.tile([P, 1], f32, tag=f"sv_{conv}")
            nc.vector.tensor_scalar_mul(sv_t[:], absmax[:], c_sv)
            sv[conv] = sv_t
            rcp = consts.tile([P, 1], f32, tag=f"rcp_{conv}")
            nc.vector.reciprocal(rcp[:], absmax[:])
            rs = consts.tile([P, 1], f32, tag=f"rs_{conv}")
            nc.vector.tensor_scalar_mul(rs[:], rcp[:], nw)

            # integer-quantize in natural [oc, ic*9] layout (per-partition scalar)
            wq = wtmp_pool.tile([P, P * NTAPS], f32, tag="wq")
            nc.vector.tensor_scalar(
                wq[:], wnat[:], rs[:, 0:1], MAGIC, op0=op.mult, op1=op.add
            )
            nc.vector.tensor_scalar(
                wq[:], wq[:], MAGIC, nw, op0=op.subtract, op1=op.min
            )
            nc.vector.tensor_scalar_max(wq[:], wq[:], -nw)
            if scale_by_ratio:
                rcp_svh = consts.tile([P, 1], f32)
                nc.vector.reciprocal(rcp_svh[:], sv["h"][:, 0:1])
                ratio = consts.tile([P, 1], f32)
                nc.vector.tensor_tensor(
                    ratio[:], sv_t[:], rcp_svh[:], op=op.mult
                )
                nc.vector.tensor_scalar_mul(wq[:], wq[:], ratio[:, 0:1])

            # transpose each 3x3 tap: [oc, ic] -> [ic, oc], cast to bf16
            qwt_t = consts.tile([P, NTAPS * P], bf16, tag=f"qwt_{conv}")
            wq_v = wq[:].rearrange("p (i t) -> p t i", t=NTAPS)
            for base in range(0, NTAPS, 4):
                n = min(4, NTAPS - base)
                tp = tp_psum.tile([P, 4 * P], f32, tag="tp")
                for j in range(n):
                    nc.tensor.transpose(
                        tp[:, j * P:(j + 1) * P],
                        wq_v[:, base + j, :], identity[:],
                    )
                nc.vector.tensor_copy(
                    out=qwt_t[:, base * P:(base + n) * P], in_=tp[:, :n * P]
                )
            qwt[conv] = qwt_t

        def mask_prep(b, xt):
            """Block sums -> threshold -> full-res {0,1} masks [P, NPIX].

            The w-blocksum reduce writes its output TRANSPOSED to (wb, h)
            order so the h-blocksum is a single contiguous-group reduce;
            the threshold result is fixed back to (hb, wb) with a tiny copy.
            (gpsimd deliberately unused here: it contends with DVE for the
            shared SBUF port.)"""
            r1 = acts.tile([P, H * NCHUNK], f32, tag="r1")   # [P, 392] (wb, h)
            nc.vector.reduce_sum(
                r1[:].rearrange("p (w h) -> p h w", w=NCHUNK),
                xt[:].rearrange("p (r c) -> p r c", c=POOL_K),
                axis=X,
            )
            r2 = acts.tile([P, NCHUNK * NCHUNK], f32, tag="r2")  # [P,49] (wb,hb)
            nc.vector.reduce_sum(
                r2[:], r1[:].rearrange("p (g c) -> p g c", c=POOL_K), axis=X
            )
            mt = acts.tile([P, NCHUNK * NCHUNK], f32, tag="mt")
            nc.vector.tensor_scalar(
                mt[:], r2[:], sum_thresh, None, op0=op.is_ge
            )
            # fix-up to (hb, wb) order with a tiny transposing copy
            m = acts.tile([P, NCHUNK * NCHUNK], f32, tag="m")
            nc.vector.tensor_copy(
                out=m[:], in_=mt[:].rearrange("p (w h) -> p h w", w=NCHUNK)
            )
            # expand to full res: [P,49] -> [P,392] (DVE) -> [P,3136] (ACT)
            mr = acts.tile([P, NCHUNK * W], f32, tag="mr")
            nc.vector.tensor_copy(
                out=mr[:].rearrange("p (r c) -> p r c", c=POOL_K),
                in_=m[:].unsqueeze(2).broadcast_to((P, NCHUNK * NCHUNK, POOL_K)),
            )
            mexp_h = maskp.tile([P, NPIX], f32, tag="mexp_h")
            mh3 = mexp_h[:].rearrange("p (r c) -> p r c", r=H)
            for hb in range(NCHUNK):
                nc.scalar.copy(
                    out=mh3[:, hb * POOL_K:(hb + 1) * POOL_K, :],
                    in_=mr[:, hb * W:(hb + 1) * W]
                    .unsqueeze(1).broadcast_to((P, POOL_K, W)),
                )
            mexp_l = maskp.tile([P, NPIX], f32, tag="mexp_l")
            nc.vector.tensor_scalar(
                mexp_l[:], mexp_h[:], -1.0, 1.0, op0=op.mult, op1=op.add
            )
            return {"h": mexp_h, "l": mexp_l}

        def quant_act(b, xt, mexp, conv, inv_s, qmax):
            """relu/scale (ACT) -> min+round (DVE) -> mask-mult into padded
            bf16 tile."""
            r = acts.tile([P, NPIX], f32, tag="r")
            nc.scalar.activation(
                r[:], xt[:], mybir.ActivationFunctionType.Relu, scale=inv_s
            )
            t = r
            nc.vector.tensor_scalar(
                t[:], r[:], qmax, MAGIC, op0=op.min, op1=op.add
            )
            q = qtiles.tile([P, NPAD], bf16, tag="q")
            q2 = q[:].rearrange("p (r c) -> p r c", r=HP)
            # zero borders: rows 0,57 and cols 0,57 (gpsimd; keeps DVE free)
            nc.gpsimd.memset(q2[:, 0:HP:HP - 1, :], 0.0)
            nc.gpsimd.memset(q2[:, 1:HP - 1, 0:WP:WP - 1], 0.0)
            nc.vector.scalar_tensor_tensor(
                out=q2[:, 1:H + 1, 1:W + 1],
                in0=t[:].rearrange("p (r c) -> p r c", r=H),
                scalar=MAGIC,
                in1=mexp[conv][:].rearrange("p (r c) -> p r c", r=H),
                op0=op.subtract, op1=op.mult,
            )
            return q

        def conv_pass(qa_tile, conv, pss, first):
            """One full conv pass (9 taps x 7 chunks) accumulating into the
            7 live PSUM banks. Low weights are pre-scaled by sv_l/sv_h so both
            passes share banks and a single evacuation."""
            q2 = qa_tile[:].rearrange("p (r c) -> p r c", r=HP)
            for c in range(NCHUNK):
                r0 = c * ROWS_PER_CHUNK
                for tap in range(NTAPS):
                    kh, kw = divmod(tap, 3)
                    rhs = q2[:, r0 + kh:r0 + kh + ROWS_PER_CHUNK, kw:kw + W]
                    nc.tensor.matmul(
                        pss[c][:], qwt[conv][:, tap * P:(tap + 1) * P], rhs,
                        start=(first and tap == 0),
                        stop=(not first and tap == NTAPS - 1),
                    )

        def conv_image(b, qa):
            acc = outs_pool.tile([P, NPIX], f32, tag="acc")
            pss = [conv_psum.tile([P, NFREE], f32, tag="ps", name=f"ps{b}_{c}")
                   for c in range(NCHUNK)]
            conv_pass(qa["h"], "h", pss, True)
            conv_pass(qa["l"], "l", pss, False)
            for c in range(NCHUNK):
                r0 = c * ROWS_PER_CHUNK
                seg = acc[:, r0 * W:(r0 + ROWS_PER_CHUNK) * W]
                if c % 2 == 0:
                    nc.scalar.mul(seg, pss[c][:], sv["h"][:, 0:1])
                else:
                    nc.vector.tensor_scalar_mul(seg, pss[c][:], sv["h"][:, 0:1])
                nc.sync.dma_start(
                    out=y_d[b][:, r0 * W:(r0 + ROWS_PER_CHUNK) * W], in_=seg
                )

        # ---------------- schedule ----------------
        # weights-h first, then image 0's high-path quant, then weights-l,
        # so image 0's convs can start while the tail of setup still runs.
        weight_prep("h", wh_d, 127.0, c_svh, False)

        xts = {}
        xts[0] = acts.tile([P, NPIX], f32, tag="xt", name="xt0")
        nc.gpsimd.dma_start(out=xts[0][:], in_=x_d[0])
        mexp0 = mask_prep(0, xts[0])
        qa0 = {"h": quant_act(0, xts[0], mexp0, "h", inv_sh, 255.0)}

        weight_prep("l", wl_d, 7.0, c_svl, True)
        qa0["l"] = quant_act(0, xts[0], mexp0, "l", inv_sl, 15.0)
        conv_image(0, qa0)

        for b in range(1, bpc):
            xt = acts.tile([P, NPIX], f32, tag="xt", name=f"xt{b}")
            nc.gpsimd.dma_start(out=xt[:], in_=x_d[b])
            mexp = mask_prep(b, xt)
            qa = {
                "h": quant_act(b, xt, mexp, "h", inv_sh, 255.0),
                "l": quant_act(b, xt, mexp, "l", inv_sl, 15.0),
            }
            conv_image(b, qa)


def make_bass(inv_sh, inv_sl, c_svh, c_svl, bpc=BPC):
    import concourse.bacc as bacc
    import concourse.mybir as mybir
    from concourse.tile import TileContext

    f32 = mybir.dt.float32
    nc = bacc.Bacc("TRN2", debug=False)
    x = nc.dram_tensor("x", [bpc, P, NPIX], f32, kind="ExternalInput")
    wh = nc.dram_tensor("w_high", [P, P * NTAPS], f32, kind="ExternalInput")
    wl = nc.dram_tensor("w_low", [P, P * NTAPS], f32, kind="ExternalInput")
    y = nc.dram_tensor("y", [bpc, P, NPIX], f32, kind="ExternalOutput")
    aps = {"x": x.ap(), "w_high": wh.ap(), "w_low": wl.ap(), "y": y.ap()}
    with TileContext(nc) as tc:
        build_program(nc, tc, aps, inv_sh, inv_sl, c_svh, c_svl, bpc=bpc)
    nc.compile()
    return nc


def _scale_consts(act_scale_high, act_scale_low):
    sh = float(np.float32(act_scale_high))
    sl = float(np.float32(act_scale_low))
    inv_sh = float(np.float32(1.0 / np.float64(sh)))
    inv_sl = float(np.float32(1.0 / np.float64(sl)))
    c_svh = float(np.float32(np.float64(sh) / 127.0))
    c_svl = float(np.float32(np.float64(sl) / 7.0))
    return inv_sh, inv_sl, c_svh, c_svl


def _run(x, w_high, w_low, act_scale_high, act_scale_low, trace=False, **kw):
    from concourse import bass_utils

    x = np.ascontiguousarray(np.asarray(x, dtype=np.float32))
    w_high = np.ascontiguousarray(np.asarray(w_high, dtype=np.float32))
    w_low = np.ascontiguousarray(np.asarray(w_low, dtype=np.float32))

    inv_sh, inv_sl, c_svh, c_svl = _scale_consts(act_scale_high, act_scale_low)
    nc = make_bass(inv_sh, inv_sl, c_svh, c_svl)

    wh_flat = w_high.reshape(P, P * NTAPS)
    wl_flat = w_low.reshape(P, P * NTAPS)
    in_maps = []
    for core in range(N_CORES):
        xs = x[core * BPC:(core + 1) * BPC].reshape(BPC, P, NPIX)
        in_maps.append(
            {
                "x": np.ascontiguousarray(xs),
                "w_high": wh_flat,
                "w_low": wl_flat,
            }
        )
    res = bass_utils.run_bass_kernel_spmd(
        nc, in_maps, core_ids=list(range(N_CORES)), trace=trace, **kw
    )
    y = np.concatenate([r["y"].reshape(BPC, P, H, W) for r in res.results], axis=0)
    return y, res


def kernel(x, w_high, w_low, act_scale_high, act_scale_low):
    y, _ = _run(x, w_high, w_low, act_scale_high, act_scale_low)
    return y
